# revision 13
# baseline (speedup 1.0000x reference)
"""Causal multi-head attention block (LN + rotary QKV + causal attention +
out-projection) on 8 Trainium2 NeuronCores.

Sharding: data-parallel over batch (b=2), tensor-parallel over heads
(16 heads -> 4 per core). Core c handles batch c//4, heads 4*(c%4)..+4.
Each core computes a partial out-projection (row-parallel w_out); the host
sums the 4 partials per batch.

Per-core pipeline (software-pipelined: tile production for chunk c+1 is
interleaved with attention for chunk c at block granularity so the PE
stays continuously busy and reaches the full 2.4 GHz p-state):
  - x arrives bf16 (LN stats only) and host-pretransposed fp32 (xT, the
    QKV stationary operand); both DMA'd in 512-token chunks.
  - LN folded around QKV: one fused scalar_tensor_tensor per matmul
    region computes raw + mu*(-colsum(W)) (colsums negated on host);
    rstd folded into the rotary cos/sin tiles and the V copy.
  - rotary on DVE (pair-interleaved features, adjacent-pair swap view).
  - attention: S^T = K_j Q_c^T blocks, trimmed to the causal range
    [q0:512]; exp on ACT; causal mask via one gpsimd affine_select on
    just the diagonal 128x128 square; PV in bf16 with a ones column on
    V so the softmax denominator comes out of the same matmul;
    normalization via DVE reciprocal + gpsimd broadcast.
  - PSUM packed into exactly 8 banks: qk[1], {v,transpose,y} shared
    ring[1], S double-buffered[4], PV accumulator[2].
"""
import sys
import os
import numpy as np
from contextlib import ExitStack

sys.path.insert(0, '/opt/trn_rl_repo')
if '/root/.axon_site' not in sys.path:
    sys.path.insert(0, '/root/.axon_site')

import concourse.bass as bass
import concourse.tile as tile
from concourse import mybir, bacc
from concourse.bass_utils import run_bass_kernel_spmd
from concourse.masks import make_identity

F32 = mybir.dt.float32
F32R = mybir.dt.float32r
BF16 = mybir.dt.bfloat16
EXPF = mybir.ActivationFunctionType.Exp
LNF = mybir.ActivationFunctionType.Ln
MUL = mybir.AluOpType.mult
ADD = mybir.AluOpType.add

N = 2048          # sequence length
D = 1024          # model dim
DH = 64           # head dim
NT = N // 128     # 16 token tiles
NCH = N // 512    # 4 q-chunks
LN_EPS = 1e-5

_cache = {}


def _patch_act_tables():
    """Keep Exp and Ln only in natural_log_exp_and_others so the table-load
    inserter can't ping-pong between exp_and_others and natural_log."""
    if _cache.get('act_patched'):
        return
    import concourse.bacc as bacc_mod
    orig = bacc_mod.get_activation_tables

    def patched(arch):
        t = dict(orig(arch))
        out = {}
        for name, fns in t.items():
            fns = set(fns)
            if name != 'natural_log_exp_and_others':
                fns.discard(mybir.ActivationFunctionType.Exp)
                fns.discard(mybir.ActivationFunctionType.Ln)
            out[name] = fns
        return out

    bacc_mod.get_activation_tables = patched
    _cache['act_patched'] = True


def _ap(t, off, dims):
    """Free-dim view of tile t at free-offset off with custom free dims."""
    return bass.AP(tensor=t.tensor, offset=t.offset + off, ap=[t.ap[0]] + dims)


def build():
    _patch_act_tables()
    nc = bacc.Bacc()
    x_d = nc.declare_dram_parameter("x", [N, D], BF16, isOutput=False)
    xT_d = nc.declare_dram_parameter("xT", [D, N], F32R, isOutput=False)
    wqk_d = nc.declare_dram_parameter("wqk", [D, 512], F32R, isOutput=False)
    wv_d = nc.declare_dram_parameter("wv", [D, 256], F32R, isOutput=False)
    wo_d = nc.declare_dram_parameter("wo", [256, D], BF16, isOutput=False)
    trig_d = nc.declare_dram_parameter("trig", [N, 2 * DH], BF16, isOutput=False)
    cqkv_d = nc.declare_dram_parameter("cqkv", [1, 768], F32, isOutput=False)
    y_d = nc.declare_dram_parameter("y", [N, D], F32, isOutput=True)

    x_r = x_d.rearrange("(j p) d -> p j d", p=128)
    xT_r = xT_d.rearrange("(k p) t -> p k t", p=128)
    y_r = y_d.rearrange("(j p) d -> p j d", p=128)

    with tile.TileContext(nc) as tc:
        with ExitStack() as cx:
            const = cx.enter_context(tc.tile_pool(name="const", bufs=1))
            xp = cx.enter_context(tc.tile_pool(name="xp", bufs=2))
            pa = cx.enter_context(tc.tile_pool(name="pa", bufs=3))
            st = cx.enter_context(tc.tile_pool(name="st", bufs=4))
            pb = cx.enter_context(tc.tile_pool(name="pb", bufs=4))
            nrm = cx.enter_context(tc.tile_pool(name="nrm", bufs=2))
            yout = cx.enter_context(tc.tile_pool(name="yout", bufs=1))
            # PSUM: exactly 8 banks
            pq = cx.enter_context(tc.tile_pool(name="pq", bufs=1, space="PSUM"))
            pm = cx.enter_context(tc.tile_pool(name="pm", bufs=1, space="PSUM"))
            ps = cx.enter_context(tc.tile_pool(name="ps", bufs=2, space="PSUM"))
            pot = cx.enter_context(tc.tile_pool(name="pot", bufs=1, space="PSUM"))

            # --- chunked input DMAs (issued up front, ring-buffered,
            # spread across engine DGE queues so transfers parallelize) ----
            x_ch, xT_ch = [], []
            wqk = const.tile([128, 8, 512], F32R)
            wv = const.tile([128, 8, 256], F32R)
            wo = const.tile([128, 2, 1024], BF16)
            trig = const.tile([128, NT, 2 * DH], BF16)
            cqkv_row = const.tile([1, 768], F32)

            def dma_x(c, eng):
                xt = xp.tile([128, 4, D], BF16, tag="x", name="x_t")
                nc.scalar.dma_start(out=xt[:], in_=x_r[:, 4 * c:4 * c + 4, :])
                x_ch.append(xt)
                xtt = xp.tile([128, 8, 512], F32R, tag="xT", name="xT_t")
                eng.dma_start(out=xtt[:], in_=xT_r[:, :, 512 * c:512 * (c + 1)])
                xT_ch.append(xtt)

            nc.gpsimd.dma_start(out=cqkv_row[:], in_=cqkv_d[:])
            nc.gpsimd.dma_start(out=trig[:],
                                in_=trig_d.rearrange("(j p) d -> p j d", p=128))
            dma_x(0, nc.sync)
            nc.sync.dma_start(out=wqk[:], in_=wqk_d.rearrange("(k p) f -> p k f", p=128))
            nc.gpsimd.dma_start(out=wv[:], in_=wv_d.rearrange("(k p) f -> p k f", p=128))
            dma_x(1, nc.sync)
            nc.gpsimd.dma_start(out=wo[:], in_=wo_d.rearrange("(g p) f -> p g f", p=128))
            dma_x(2, nc.sync)
            dma_x(3, nc.sync)

            ident = const.tile([128, 128], F32)
            make_identity(nc, ident[:])
            eps_t = const.tile([128, 1], F32)
            nc.vector.memset(eps_t[:], LN_EPS)
            cqkv_b = const.tile([128, 768], F32)
            nc.gpsimd.partition_broadcast(cqkv_b[:], cqkv_row[:])

            # persistent activations
            qT = const.tile([128, 2, N], BF16)     # [2 heads x 64 d, pair, tok]
            kT = const.tile([128, 2, N], BF16)
            vA = const.tile([128, NT, 4, DH + 1], BF16)   # V_ext, ones col 64
            oT = const.tile([128, 2, N], BF16)     # attention out^T per pair
            nc.gpsimd.memset(vA[:, :, :, DH:DH + 1], 1.0)

            rot = {}     # tile j -> qk_rot handle (consumed by transposes)

            # ---------------- tile production -----------------------------
            def emit_tp(jp):
                """Transpose tile jp's rotated q/k into qT/kT."""
                sl = slice(128 * jp, 128 * (jp + 1))
                tp = pm.tile([128, 4, 128], F32, tag="m", name="tp")
                for f in range(4):
                    nc.tensor.transpose(tp[:, f, :],
                                        rot[jp][:, 128 * f:128 * (f + 1)], ident[:])
                nc.vector.tensor_copy(out=qT[:, :, sl], in_=tp[:, 0:2, :])
                nc.scalar.copy(out=kT[:, :, sl], in_=tp[:, 2:4, :])
                del rot[jp]

            def produce(j):
                c, jl = j // 4, j % 4
                if j >= 1:
                    emit_tp(j - 1)
                x_t = x_ch[c]
                xT_t = xT_ch[c]
                qk_ps = pq.tile([128, 512], F32, tag="qk", name="qkp")
                for k in range(8):
                    nc.tensor.matmul(qk_ps[:], xT_t[:, k, 128 * jl:128 * (jl + 1)],
                                     wqk[:, k, :], start=(k == 0), stop=(k == 7))
                v_ps = pm.tile([128, 256], F32, tag="m", name="vp")
                for k in range(8):
                    nc.tensor.matmul(v_ps[:], xT_t[:, k, 128 * jl:128 * (jl + 1)],
                                     wv[:, k, :], start=(k == 0), stop=(k == 7))
                stats = st.tile([128, 2, 6], F32, tag="stats", name="stats")
                nc.vector.bn_stats(out=stats[:, 0, :], in_=x_t[:, jl, 0:512])
                nc.vector.bn_stats(out=stats[:, 1, :], in_=x_t[:, jl, 512:1024])
                mv = st.tile([128, 2], F32, tag="mv", name="mv")
                nc.vector.bn_aggr(out=mv[:], in_=stats[:])
                # rstd = exp(-0.5*ln(var+eps))
                lnv = st.tile([128, 1], F32, tag="lnv", name="lnv")
                nc.scalar.activation(out=lnv[:], in_=mv[:, 1:2], func=LNF, bias=eps_t[:])
                rstd = st.tile([128, 1], F32, tag="rstd", name="rstd")
                nc.scalar.activation(out=rstd[:], in_=lnv[:], func=EXPF, scale=-0.5)
                # LN correction fused: qkv_c = raw + mu*(-colsum) (cqkv negated)
                qkc = pa.tile([128, 512], BF16, tag="qkc", name="qkc")
                nc.vector.scalar_tensor_tensor(
                    out=qkc[:], in0=cqkv_b[:, 0:512], scalar=mv[:, 0:1],
                    in1=qk_ps[:], op0=MUL, op1=ADD)
                vc = pa.tile([128, 256], BF16, tag="vc", name="vc")
                nc.vector.scalar_tensor_tensor(
                    out=vc[:], in0=cqkv_b[:, 512:768], scalar=mv[:, 0:1],
                    in1=v_ps[:], op0=MUL, op1=ADD)
                nc.gpsimd.tensor_scalar(
                    out=vA[:, j, :, 0:DH],
                    in0=vc[:].rearrange("p (h d) -> p h d", d=DH),
                    scalar1=rstd[:], scalar2=None, op0=MUL)
                # rstd-scaled rotary coefficient tiles (cos|sin packed)
                css = st.tile([128, 2 * DH], BF16, tag="css", name="css")
                nc.vector.tensor_scalar(out=css[:], in0=trig[:, j, :],
                                        scalar1=rstd[:], scalar2=None, op0=MUL)
                # rotary: qk_rot = qkc*cos + swap_adj(qkc)*sin
                cos_b = _ap(css, 0, [[0, 8], [1, DH]])
                sin_b = _ap(css, DH, [[0, 8], [2, 32], [1, 2]])
                t_cos = pa.tile([128, 512], BF16, tag="tcos", name="tcos")
                nc.vector.tensor_tensor(
                    out=t_cos[:].rearrange("p (g d) -> p g d", d=DH),
                    in0=qkc[:].rearrange("p (g d) -> p g d", d=DH),
                    in1=cos_b, op=MUL)
                t_sin = pa.tile([128, 512], BF16, tag="tsin", name="tsin")
                qk_swap = _ap(qkc, 1, [[DH, 8], [2, 32], [-1, 2]])
                nc.vector.tensor_tensor(
                    out=t_sin[:].rearrange("p (g i t) -> p g i t", g=8, t=2),
                    in0=qk_swap, in1=sin_b, op=MUL)
                qk_rot = pa.tile([128, 512], F32, tag="qkr", name="qkr")
                nc.vector.tensor_tensor(out=qk_rot[:], in0=t_cos[:], in1=t_sin[:],
                                        op=ADD)
                rot[j] = qk_rot

            # ---------------- attention closures --------------------------
            def attention_closures(c):
                """List of closures: per-block S/exp/mask/PV plus a
                normalize closure at the end of each head-pair."""
                njb = 4 * c + 4
                items = []
                for hp in range(2):
                    hst = {}

                    def emit_pv(pj, pt, hp=hp, hst=hst, njb=njb, c=c):
                        pq0 = max(0, 128 * (pj - 4 * c))
                        otp = hst['ot']
                        for hh in range(2):
                            nc.tensor.matmul(
                                otp[:, 512 * hh + pq0:512 * (hh + 1)],
                                vA[:, pj, 2 * hp + hh, :],
                                pt[:, hh, pq0:512],
                                start=(pj == 0), stop=(pj == njb - 1),
                                skip_group_check=True)

                    def blk(jj, hp=hp, hst=hst, c=c, emit_pv=emit_pv):
                        if jj == 0:
                            hst['ot'] = pot.tile([DH + 1, 1024], F32, tag="ot",
                                                 name="ot")
                            hst['pend'] = []
                        dj = jj - 4 * c
                        q0 = max(0, 128 * dj)
                        s_ps = ps.tile([128, 2, 512], F32, tag="s", name="s")
                        for hh in range(2):
                            bp = 64 * hh
                            nc.tensor.matmul(
                                s_ps[:, hh, q0:512],
                                kT[bp:bp + 64, hp, 128 * jj:128 * (jj + 1)],
                                qT[bp:bp + 64, hp, 512 * c + q0:512 * (c + 1)],
                                start=True, stop=True, skip_group_check=True)
                        p_t = pb.tile([128, 2, 512], BF16, tag="p", name="p")
                        if dj < 0:
                            nc.scalar.activation(out=p_t[:], in_=s_ps[:], func=EXPF)
                        else:
                            nc.scalar.activation(out=p_t[:, :, q0:512],
                                                 in_=s_ps[:, :, q0:512], func=EXPF)
                            nc.gpsimd.affine_select(
                                out=p_t[:, :, q0:q0 + 128],
                                in_=p_t[:, :, q0:q0 + 128],
                                compare_op=mybir.AluOpType.is_ge,
                                fill=0.0, base=0,
                                pattern=[[0, 2], [1, 128]], channel_multiplier=-1)
                        hst['pend'].append((jj, p_t))
                        if len(hst['pend']) > 2:
                            emit_pv(*hst['pend'].pop(0))

                    def norm(hp=hp, hst=hst, c=c, emit_pv=emit_pv):
                        while hst['pend']:
                            emit_pv(*hst['pend'].pop(0))
                        otp = hst['ot']
                        # copy PSUM->SBUF first (split engines) so the ot
                        # slot frees fast; normalize off the critical path
                        oraw = nrm.tile([DH + 1, 1024], F32, tag="oraw",
                                        name="oraw")
                        nc.scalar.copy(out=oraw[:, 0:512], in_=otp[:, 0:512])
                        nc.vector.tensor_copy(out=oraw[:, 512:1024],
                                              in_=otp[:, 512:1024])
                        lnl = nrm.tile([1, 1024], F32, tag="lnl", name="lnl")
                        nc.scalar.activation(out=lnl[:], in_=oraw[DH:DH + 1, :],
                                             func=LNF)
                        rec = nrm.tile([1, 1024], F32, tag="rec", name="rec")
                        nc.scalar.activation(out=rec[:], in_=lnl[:], func=EXPF,
                                             scale=-1.0)
                        rec_b = nrm.tile([64, 1024], F32, tag="recb", name="recb")
                        nc.gpsimd.partition_broadcast(rec_b[:], rec[:])
                        for hh in range(2):
                            nc.gpsimd.tensor_tensor(
                                out=oT[64 * hh:64 * (hh + 1), hp,
                                       512 * c:512 * (c + 1)],
                                in0=oraw[0:DH, 512 * hh:512 * (hh + 1)],
                                in1=rec_b[:, 512 * hh:512 * (hh + 1)],
                                op=MUL)

                    for jj in range(njb):
                        items.append(lambda jj=jj, blk=blk: blk(jj))
                    items.append(norm)
                return items

            # ---------------- out-projection ------------------------------
            def outproj(c):
                # the last chunk's out-projection runs after all attention:
                # draw its psum from the then-idle double-buffered s ring so
                # matmul/copy overlap instead of serializing on the misc slot
                ypool, ytag = (ps, "s") if c == 3 else (pm, "m")
                ysb = yout.tile([128, 4, 1024], F32, tag="ysb", name="ysb")
                for jl in range(4):
                    j = 4 * c + jl
                    for m in range(2):
                        y_ps = ypool.tile([128, 512], F32, tag=ytag, name="yp")
                        for hp2 in range(2):
                            nc.tensor.matmul(y_ps[:],
                                             oT[:, hp2, 128 * j:128 * (j + 1)],
                                             wo[:, hp2, 512 * m:512 * (m + 1)],
                                             start=(hp2 == 0), stop=(hp2 == 1))
                        dst = ysb[:, jl, 512 * m:512 * (m + 1)]
                        if (jl + m) % 2 == 0:
                            nc.vector.tensor_copy(out=dst, in_=y_ps[:])
                        else:
                            nc.scalar.copy(out=dst, in_=y_ps[:])
                nc.sync.dma_start(out=y_r[:, 4 * c:4 * c + 4, :], in_=ysb[:])

            # ---------------- pipelined emission --------------------------
            for step in range(5):
                cprev = step - 1
                blocks = attention_closures(cprev) if cprev >= 0 else []
                bidx = 0

                def drain(n):
                    nonlocal bidx
                    for _ in range(n):
                        if bidx < len(blocks):
                            blocks[bidx]()
                            bidx += 1

                if step < 4:
                    quarter = (len(blocks) + 3) // 4 if blocks else 0
                    for i, j in enumerate(range(4 * step, 4 * step + 4)):
                        produce(j)
                        if i == 1 and step >= 2:
                            outproj(step - 2)
                        drain(quarter)
                    drain(len(blocks))
                else:
                    emit_tp(15)
                    outproj(2)
                    drain(len(blocks))
            outproj(3)

    nc.finalize()
    return nc


def _host_shards(x, rotary_pos_emb, ln_w, ln_b, w_qkv, w_out):
    """Build the 8 per-core input maps."""
    import ml_dtypes
    BF = ml_dtypes.bfloat16
    SCALE = DH ** -0.5
    # pair-interleaved feature order within each head: (i, i+32) adjacent
    perm = np.empty(DH, dtype=np.int64)
    perm[0::2] = np.arange(32)
    perm[1::2] = np.arange(32) + 32
    cos = np.cos(rotary_pos_emb).astype(np.float32)     # [N, DH]
    sin = np.sin(rotary_pos_emb).astype(np.float32)
    cosn = cos[:, perm]
    sinn = sin[:, perm].copy()
    sinn[:, 0::2] *= -1.0                               # -sin on even slots
    trig = np.ascontiguousarray(np.concatenate([cosn, sinn], axis=1)).astype(BF)

    lw = np.asarray(ln_w, dtype=np.float32)[:, None]
    w_q = (np.asarray(w_qkv[:, 0:1024]) * SCALE * lw).astype(np.float32)
    w_k = (np.asarray(w_qkv[:, 1024:2048]) * lw).astype(np.float32)
    w_v = (np.asarray(w_qkv[:, 2048:3072]) * lw).astype(np.float32)
    if np.abs(np.asarray(ln_b)).max() != 0:
        raise NotImplementedError("nonzero ln_b not supported by this kernel")

    in_maps = []
    for core in range(8):
        bi = core // 4
        h0 = 4 * (core % 4)
        qcols = [w_q[:, DH * (h0 + h):DH * (h0 + h + 1)][:, perm] for h in range(4)]
        kcols = [w_k[:, DH * (h0 + h):DH * (h0 + h + 1)][:, perm] for h in range(4)]
        wqk = np.ascontiguousarray(np.concatenate(qcols + kcols, axis=1))
        wv = np.ascontiguousarray(w_v[:, DH * h0:DH * (h0 + 4)])
        wo = np.ascontiguousarray(
            np.asarray(w_out)[DH * h0:DH * (h0 + 4), :]).astype(BF)
        xb = np.ascontiguousarray(np.asarray(x[bi])).astype(np.float32)
        # correction colsums NEGATED so the fused stt is (c*mu + raw)
        cq = -np.concatenate([wqk.sum(axis=0), wv.sum(axis=0)]).astype(np.float32)
        in_maps.append({
            "x": np.ascontiguousarray(xb).astype(BF),
            "xT": np.ascontiguousarray(xb.T),
            "wqk": wqk, "wv": wv, "wo": wo,
            "trig": trig,
            "cqkv": np.ascontiguousarray(cq[None, :]),
        })
    return in_maps


def run(inputs, trace=False):
    if 'nc' not in _cache:
        _cache['nc'] = build()
    nc = _cache['nc']
    in_maps = _host_shards(**inputs)
    res = run_bass_kernel_spmd(nc, in_maps, core_ids=list(range(8)), trace=trace)
    parts = [res.results[i]["y"] for i in range(8)]
    y = np.stack([
        parts[0] + parts[1] + parts[2] + parts[3],
        parts[4] + parts[5] + parts[6] + parts[7],
    ]).astype(np.float32)
    return y, res


def kernel(**inputs):
    y, _ = run(inputs, trace=False)
    return y


# revision 15
# speedup vs baseline: 1.0677x; 1.0677x over previous
"""Causal multi-head attention block (LN + rotary QKV + causal attention +
out-projection) on 8 Trainium2 NeuronCores.

Sharding: data-parallel over batch (b=2), tensor-parallel over heads
(16 heads -> 4 per core). Core c handles batch c//4, heads 4*(c%4)..+4.
Each core computes a partial out-projection (row-parallel w_out); the host
sums the 4 partials per batch.

Per-core pipeline (software-pipelined: tile production for chunk c+1 is
interleaved with attention for chunk c at block granularity so the PE
stays continuously busy and reaches the full 2.4 GHz p-state):
  - x arrives bf16 (LN stats only) and host-pretransposed fp32 (xT, the
    QKV stationary operand); both DMA'd in 512-token chunks.
  - LN folded around QKV: one fused scalar_tensor_tensor per matmul
    region computes raw + mu*(-colsum(W)) (colsums negated on host);
    rstd folded into the rotary cos/sin tiles and the V copy.
  - rotary on DVE (pair-interleaved features, adjacent-pair swap view).
  - attention: S^T = K_j Q_c^T blocks, trimmed to the causal range
    [q0:512]; exp on ACT; causal mask via one gpsimd affine_select on
    just the diagonal 128x128 square; PV in bf16 with a ones column on
    V so the softmax denominator comes out of the same matmul;
    normalization via DVE reciprocal + gpsimd broadcast.
  - PSUM packed into exactly 8 banks: qk[1], {v,transpose,y} shared
    ring[1], S double-buffered[4], PV accumulator[2].
"""
import sys
import os
import numpy as np
from contextlib import ExitStack

sys.path.insert(0, '/opt/trn_rl_repo')
if '/root/.axon_site' not in sys.path:
    sys.path.insert(0, '/root/.axon_site')

import concourse.bass as bass
import concourse.tile as tile
from concourse import mybir, bacc
from concourse.bass_utils import run_bass_kernel_spmd
from concourse.masks import make_identity

F32 = mybir.dt.float32
F32R = mybir.dt.float32r
BF16 = mybir.dt.bfloat16
EXPF = mybir.ActivationFunctionType.Exp
LNF = mybir.ActivationFunctionType.Ln
MUL = mybir.AluOpType.mult
ADD = mybir.AluOpType.add

N = 2048          # sequence length
D = 1024          # model dim
DH = 64           # head dim
NT = N // 128     # 16 token tiles
NCH = N // 512    # 4 q-chunks
LN_EPS = 1e-5

_cache = {}


def _patch_act_tables():
    """Keep Exp and Ln only in natural_log_exp_and_others so the table-load
    inserter can't ping-pong between exp_and_others and natural_log."""
    if _cache.get('act_patched'):
        return
    import concourse.bacc as bacc_mod
    orig = bacc_mod.get_activation_tables

    def patched(arch):
        t = dict(orig(arch))
        out = {}
        for name, fns in t.items():
            fns = set(fns)
            if name != 'natural_log_exp_and_others':
                fns.discard(mybir.ActivationFunctionType.Exp)
                fns.discard(mybir.ActivationFunctionType.Ln)
            out[name] = fns
        return out

    bacc_mod.get_activation_tables = patched
    _cache['act_patched'] = True


def _ap(t, off, dims):
    """Free-dim view of tile t at free-offset off with custom free dims."""
    return bass.AP(tensor=t.tensor, offset=t.offset + off, ap=[t.ap[0]] + dims)


def build():
    _patch_act_tables()
    nc = bacc.Bacc()
    x_d = nc.declare_dram_parameter("x", [N, D], BF16, isOutput=False)
    xT_d = nc.declare_dram_parameter("xT", [D, N], F32R, isOutput=False)
    wqk_d = nc.declare_dram_parameter("wqk", [D, 512], F32R, isOutput=False)
    wv_d = nc.declare_dram_parameter("wv", [D, 256], F32R, isOutput=False)
    wo_d = nc.declare_dram_parameter("wo", [256, D], BF16, isOutput=False)
    trig_d = nc.declare_dram_parameter("trig", [N, 2 * DH], F32, isOutput=False)
    cqkv_d = nc.declare_dram_parameter("cqkv", [1, 768], F32, isOutput=False)
    y_d = nc.declare_dram_parameter("y", [N, D], F32, isOutput=True)

    x_r = x_d.rearrange("(j p) d -> p j d", p=128)
    xT_r = xT_d.rearrange("(k p) t -> p k t", p=128)
    y_r = y_d.rearrange("(j p) d -> p j d", p=128)

    with tile.TileContext(nc) as tc:
        with ExitStack() as cx:
            const = cx.enter_context(tc.tile_pool(name="const", bufs=1))
            xp = cx.enter_context(tc.tile_pool(name="xp", bufs=2))
            pa = cx.enter_context(tc.tile_pool(name="pa", bufs=2))
            st = cx.enter_context(tc.tile_pool(name="st", bufs=4))
            pb = cx.enter_context(tc.tile_pool(name="pb", bufs=4))
            nrm = cx.enter_context(tc.tile_pool(name="nrm", bufs=2))
            yout = cx.enter_context(tc.tile_pool(name="yout", bufs=1))
            # PSUM: exactly 8 banks
            pq = cx.enter_context(tc.tile_pool(name="pq", bufs=1, space="PSUM"))
            pm = cx.enter_context(tc.tile_pool(name="pm", bufs=1, space="PSUM"))
            ps = cx.enter_context(tc.tile_pool(name="ps", bufs=2, space="PSUM"))
            pot = cx.enter_context(tc.tile_pool(name="pot", bufs=1, space="PSUM"))

            # --- chunked input DMAs (issued up front, ring-buffered,
            # spread across engine DGE queues so transfers parallelize) ----
            x_ch, xT_ch = [], []
            wqk = const.tile([128, 8, 512], F32R)
            wv = const.tile([128, 8, 256], F32R)
            wo = const.tile([128, 2, 1024], BF16)
            trig = const.tile([128, NT, 2 * DH], F32)
            cqkv_row = const.tile([1, 768], F32)

            def dma_x(c, eng):
                xt = xp.tile([128, 4, D], BF16, tag="x", name="x_t")
                nc.scalar.dma_start(out=xt[:], in_=x_r[:, 4 * c:4 * c + 4, :])
                x_ch.append(xt)
                xtt = xp.tile([128, 8, 512], F32R, tag="xT", name="xT_t")
                eng.dma_start(out=xtt[:], in_=xT_r[:, :, 512 * c:512 * (c + 1)])
                xT_ch.append(xtt)

            nc.gpsimd.dma_start(out=cqkv_row[:], in_=cqkv_d[:])
            nc.gpsimd.dma_start(out=trig[:],
                                in_=trig_d.rearrange("(j p) d -> p j d", p=128))
            dma_x(0, nc.sync)
            nc.sync.dma_start(out=wqk[:], in_=wqk_d.rearrange("(k p) f -> p k f", p=128))
            nc.gpsimd.dma_start(out=wv[:], in_=wv_d.rearrange("(k p) f -> p k f", p=128))
            dma_x(1, nc.sync)
            nc.gpsimd.dma_start(out=wo[:], in_=wo_d.rearrange("(g p) f -> p g f", p=128))
            dma_x(2, nc.sync)
            dma_x(3, nc.sync)

            ident = const.tile([128, 128], F32)
            make_identity(nc, ident[:])
            eps_t = const.tile([128, 1], F32)
            nc.vector.memset(eps_t[:], LN_EPS)
            cqkv_b = const.tile([128, 768], F32)
            nc.gpsimd.partition_broadcast(cqkv_b[:], cqkv_row[:])

            # persistent activations
            qT = const.tile([128, 2, N], BF16)     # [2 heads x 64 d, pair, tok]
            kT = const.tile([128, 2, N], BF16)
            vA = const.tile([128, NT, 4, DH + 1], BF16)   # V_ext, ones col 64
            oT = const.tile([128, 2, N], BF16)     # attention out^T per pair
            nc.gpsimd.memset(vA[:, :, :, DH:DH + 1], 1.0)

            rot = {}     # tile j -> qk_rot handle (consumed by transposes)

            # ---------------- tile production -----------------------------
            def emit_tp(jp):
                """Transpose tile jp's rotated q/k into qT/kT."""
                sl = slice(128 * jp, 128 * (jp + 1))
                tp = pm.tile([128, 4, 128], F32, tag="m", name="tp")
                for f in range(4):
                    nc.tensor.transpose(tp[:, f, :],
                                        rot[jp][:, 128 * f:128 * (f + 1)], ident[:])
                nc.vector.tensor_copy(out=qT[:, :, sl], in_=tp[:, 0:2, :])
                nc.scalar.copy(out=kT[:, :, sl], in_=tp[:, 2:4, :])
                del rot[jp]

            def produce(j):
                c, jl = j // 4, j % 4
                if j >= 1:
                    emit_tp(j - 1)
                x_t = x_ch[c]
                xT_t = xT_ch[c]
                qk_ps = pq.tile([128, 512], F32, tag="qk", name="qkp")
                for k in range(8):
                    nc.tensor.matmul(qk_ps[:], xT_t[:, k, 128 * jl:128 * (jl + 1)],
                                     wqk[:, k, :], start=(k == 0), stop=(k == 7))
                v_ps = pm.tile([128, 256], F32, tag="m", name="vp")
                for k in range(8):
                    nc.tensor.matmul(v_ps[:], xT_t[:, k, 128 * jl:128 * (jl + 1)],
                                     wv[:, k, :], start=(k == 0), stop=(k == 7))
                stats = st.tile([128, 2, 6], F32, tag="stats", name="stats")
                nc.vector.bn_stats(out=stats[:, 0, :], in_=x_t[:, jl, 0:512])
                nc.vector.bn_stats(out=stats[:, 1, :], in_=x_t[:, jl, 512:1024])
                mv = st.tile([128, 2], F32, tag="mv", name="mv")
                nc.vector.bn_aggr(out=mv[:], in_=stats[:])
                # rstd = exp(-0.5*ln(var+eps))
                lnv = st.tile([128, 1], F32, tag="lnv", name="lnv")
                nc.scalar.activation(out=lnv[:], in_=mv[:, 1:2], func=LNF, bias=eps_t[:])
                rstd = st.tile([128, 1], F32, tag="rstd", name="rstd")
                nc.scalar.activation(out=rstd[:], in_=lnv[:], func=EXPF, scale=-0.5)
                # LN correction fused: qkv_c = raw + mu*(-colsum) (cqkv negated)
                qkc = pa.tile([128, 512], F32, tag="qkc", name="qkc")
                nc.vector.scalar_tensor_tensor(
                    out=qkc[:], in0=cqkv_b[:, 0:512], scalar=mv[:, 0:1],
                    in1=qk_ps[:], op0=MUL, op1=ADD)
                vc = pa.tile([128, 256], F32, tag="vc", name="vc")
                nc.vector.scalar_tensor_tensor(
                    out=vc[:], in0=cqkv_b[:, 512:768], scalar=mv[:, 0:1],
                    in1=v_ps[:], op0=MUL, op1=ADD)
                nc.vector.tensor_scalar(
                    out=vA[:, j, :, 0:DH],
                    in0=vc[:].rearrange("p (h d) -> p h d", d=DH),
                    scalar1=rstd[:], scalar2=None, op0=MUL)
                # rstd-scaled rotary coefficient tiles (cos|sin packed)
                css = st.tile([128, 2 * DH], F32, tag="css", name="css")
                nc.vector.tensor_scalar(out=css[:], in0=trig[:, j, :],
                                        scalar1=rstd[:], scalar2=None, op0=MUL)
                # rotary: qk_rot = qkc*cos + swap_adj(qkc)*sin
                cos_b = _ap(css, 0, [[0, 8], [1, DH]])
                sin_b = _ap(css, DH, [[0, 8], [2, 32], [1, 2]])
                t_cos = pa.tile([128, 512], F32, tag="tcos", name="tcos")
                nc.vector.tensor_tensor(
                    out=t_cos[:].rearrange("p (g d) -> p g d", d=DH),
                    in0=qkc[:].rearrange("p (g d) -> p g d", d=DH),
                    in1=cos_b, op=MUL)
                t_sin = pa.tile([128, 512], F32, tag="tsin", name="tsin")
                qk_swap = _ap(qkc, 1, [[DH, 8], [2, 32], [-1, 2]])
                nc.vector.tensor_tensor(
                    out=t_sin[:].rearrange("p (g i t) -> p g i t", g=8, t=2),
                    in0=qk_swap, in1=sin_b, op=MUL)
                qk_rot = pa.tile([128, 512], F32, tag="qkr", name="qkr")
                nc.vector.tensor_tensor(out=qk_rot[:], in0=t_cos[:], in1=t_sin[:],
                                        op=ADD)
                rot[j] = qk_rot

            # ---------------- attention closures --------------------------
            def attention_closures(c):
                """List of closures: per-block S/exp/mask/PV plus a
                normalize closure at the end of each head-pair."""
                njb = 4 * c + 4
                items = []
                for hp in range(2):
                    hst = {}

                    def emit_pv(pj, pt, hp=hp, hst=hst, njb=njb, c=c):
                        pq0 = max(0, 128 * (pj - 4 * c))
                        otp = hst['ot']
                        for hh in range(2):
                            nc.tensor.matmul(
                                otp[:, 512 * hh + pq0:512 * (hh + 1)],
                                vA[:, pj, 2 * hp + hh, :],
                                pt[:, hh, pq0:512],
                                start=(pj == 0), stop=(pj == njb - 1),
                                skip_group_check=True)

                    def blk(jj, hp=hp, hst=hst, c=c, emit_pv=emit_pv):
                        if jj == 0:
                            hst['ot'] = pot.tile([DH + 1, 1024], F32, tag="ot",
                                                 name="ot")
                            hst['pend'] = []
                        dj = jj - 4 * c
                        q0 = max(0, 128 * dj)
                        s_ps = ps.tile([128, 2, 512], F32, tag="s", name="s")
                        for hh in range(2):
                            bp = 64 * hh
                            nc.tensor.matmul(
                                s_ps[:, hh, q0:512],
                                kT[bp:bp + 64, hp, 128 * jj:128 * (jj + 1)],
                                qT[bp:bp + 64, hp, 512 * c + q0:512 * (c + 1)],
                                start=True, stop=True, skip_group_check=True)
                        p_t = pb.tile([128, 2, 512], BF16, tag="p", name="p")
                        if dj < 0:
                            nc.scalar.activation(out=p_t[:], in_=s_ps[:], func=EXPF)
                        else:
                            nc.scalar.activation(out=p_t[:, :, q0:512],
                                                 in_=s_ps[:, :, q0:512], func=EXPF)
                            nc.gpsimd.affine_select(
                                out=p_t[:, :, q0:q0 + 128],
                                in_=p_t[:, :, q0:q0 + 128],
                                compare_op=mybir.AluOpType.is_ge,
                                fill=0.0, base=0,
                                pattern=[[0, 2], [1, 128]], channel_multiplier=-1)
                        hst['pend'].append((jj, p_t))
                        if len(hst['pend']) > 2:
                            emit_pv(*hst['pend'].pop(0))

                    def norm(hp=hp, hst=hst, c=c, emit_pv=emit_pv):
                        while hst['pend']:
                            emit_pv(*hst['pend'].pop(0))
                        otp = hst['ot']
                        # copy PSUM->SBUF first (split engines) so the ot
                        # slot frees fast; normalize off the critical path
                        oraw = nrm.tile([DH + 1, 1024], F32, tag="oraw",
                                        name="oraw")
                        nc.scalar.copy(out=oraw[:, 0:512], in_=otp[:, 0:512])
                        nc.vector.tensor_copy(out=oraw[:, 512:1024],
                                              in_=otp[:, 512:1024])
                        lnl = nrm.tile([1, 1024], F32, tag="lnl", name="lnl", bufs=1)
                        nc.scalar.activation(out=lnl[:], in_=oraw[DH:DH + 1, :],
                                             func=LNF)
                        rec = nrm.tile([1, 1024], F32, tag="rec", name="rec", bufs=1)
                        nc.scalar.activation(out=rec[:], in_=lnl[:], func=EXPF,
                                             scale=-1.0)
                        rec_b = nrm.tile([64, 1024], F32, tag="recb", name="recb", bufs=1)
                        nc.gpsimd.partition_broadcast(rec_b[:], rec[:])
                        for hh in range(2):
                            nc.gpsimd.tensor_tensor(
                                out=oT[64 * hh:64 * (hh + 1), hp,
                                       512 * c:512 * (c + 1)],
                                in0=oraw[0:DH, 512 * hh:512 * (hh + 1)],
                                in1=rec_b[:, 512 * hh:512 * (hh + 1)],
                                op=MUL)

                    for jj in range(njb):
                        items.append(lambda jj=jj, blk=blk: blk(jj))
                    items.append(norm)
                return items

            # ---------------- out-projection ------------------------------
            def outproj(c):
                # the last chunk's out-projection runs after all attention:
                # draw its psum from the then-idle double-buffered s ring so
                # matmul/copy overlap instead of serializing on the misc slot
                ypool, ytag = (ps, "s") if c == 3 else (pm, "m")
                ysb = yout.tile([128, 4, 1024], F32, tag="ysb", name="ysb")
                for jl in range(4):
                    j = 4 * c + jl
                    for m in range(2):
                        y_ps = ypool.tile([128, 512], F32, tag=ytag, name="yp")
                        for hp2 in range(2):
                            nc.tensor.matmul(y_ps[:],
                                             oT[:, hp2, 128 * j:128 * (j + 1)],
                                             wo[:, hp2, 512 * m:512 * (m + 1)],
                                             start=(hp2 == 0), stop=(hp2 == 1))
                        dst = ysb[:, jl, 512 * m:512 * (m + 1)]
                        if (jl + m) % 2 == 0:
                            nc.vector.tensor_copy(out=dst, in_=y_ps[:])
                        else:
                            nc.scalar.copy(out=dst, in_=y_ps[:])
                nc.sync.dma_start(out=y_r[:, 4 * c:4 * c + 4, :], in_=ysb[:])

            # ---------------- pipelined emission --------------------------
            for step in range(5):
                cprev = step - 1
                blocks = attention_closures(cprev) if cprev >= 0 else []
                bidx = 0

                def drain(n):
                    nonlocal bidx
                    for _ in range(n):
                        if bidx < len(blocks):
                            blocks[bidx]()
                            bidx += 1

                if step < 4:
                    quarter = (len(blocks) + 3) // 4 if blocks else 0
                    for i, j in enumerate(range(4 * step, 4 * step + 4)):
                        produce(j)
                        if i == 1 and step >= 2:
                            outproj(step - 2)
                        drain(quarter)
                    drain(len(blocks))
                else:
                    emit_tp(15)
                    outproj(2)
                    drain(len(blocks))
            outproj(3)

    nc.finalize()
    return nc


def _host_shards(x, rotary_pos_emb, ln_w, ln_b, w_qkv, w_out):
    """Build the 8 per-core input maps."""
    import ml_dtypes
    BF = ml_dtypes.bfloat16
    SCALE = DH ** -0.5
    # pair-interleaved feature order within each head: (i, i+32) adjacent
    perm = np.empty(DH, dtype=np.int64)
    perm[0::2] = np.arange(32)
    perm[1::2] = np.arange(32) + 32
    cos = np.cos(rotary_pos_emb).astype(np.float32)     # [N, DH]
    sin = np.sin(rotary_pos_emb).astype(np.float32)
    cosn = cos[:, perm]
    sinn = sin[:, perm].copy()
    sinn[:, 0::2] *= -1.0                               # -sin on even slots
    trig = np.ascontiguousarray(np.concatenate([cosn, sinn], axis=1))

    lw = np.asarray(ln_w, dtype=np.float32)[:, None]
    w_q = (np.asarray(w_qkv[:, 0:1024]) * SCALE * lw).astype(np.float32)
    w_k = (np.asarray(w_qkv[:, 1024:2048]) * lw).astype(np.float32)
    w_v = (np.asarray(w_qkv[:, 2048:3072]) * lw).astype(np.float32)
    if np.abs(np.asarray(ln_b)).max() != 0:
        raise NotImplementedError("nonzero ln_b not supported by this kernel")

    in_maps = []
    for core in range(8):
        bi = core // 4
        h0 = 4 * (core % 4)
        qcols = [w_q[:, DH * (h0 + h):DH * (h0 + h + 1)][:, perm] for h in range(4)]
        kcols = [w_k[:, DH * (h0 + h):DH * (h0 + h + 1)][:, perm] for h in range(4)]
        wqk = np.ascontiguousarray(np.concatenate(qcols + kcols, axis=1))
        wv = np.ascontiguousarray(w_v[:, DH * h0:DH * (h0 + 4)])
        wo = np.ascontiguousarray(
            np.asarray(w_out)[DH * h0:DH * (h0 + 4), :]).astype(BF)
        xb = np.ascontiguousarray(np.asarray(x[bi])).astype(np.float32)
        # correction colsums NEGATED so the fused stt is (c*mu + raw)
        cq = -np.concatenate([wqk.sum(axis=0), wv.sum(axis=0)]).astype(np.float32)
        in_maps.append({
            "x": np.ascontiguousarray(xb).astype(BF),
            "xT": np.ascontiguousarray(xb.T),
            "wqk": wqk, "wv": wv, "wo": wo,
            "trig": trig,
            "cqkv": np.ascontiguousarray(cq[None, :]),
        })
    return in_maps


def run(inputs, trace=False):
    if 'nc' not in _cache:
        _cache['nc'] = build()
    nc = _cache['nc']
    in_maps = _host_shards(**inputs)
    res = run_bass_kernel_spmd(nc, in_maps, core_ids=list(range(8)), trace=trace)
    parts = [res.results[i]["y"] for i in range(8)]
    y = np.stack([
        parts[0] + parts[1] + parts[2] + parts[3],
        parts[4] + parts[5] + parts[6] + parts[7],
    ]).astype(np.float32)
    return y, res


def kernel(**inputs):
    y, _ = run(inputs, trace=False)
    return y


# revision 22
# speedup vs baseline: 1.0707x; 1.0029x over previous
"""Causal multi-head attention block (LN + rotary QKV + causal attention +
out-projection) on 8 Trainium2 NeuronCores.

Sharding: data-parallel over batch (b=2), tensor-parallel over heads
(16 heads -> 4 per core). Core c handles batch c//4, heads 4*(c%4)..+4.
Each core computes a partial out-projection (row-parallel w_out); the host
sums the 4 partials per batch.

Per-core pipeline (software-pipelined: tile production for chunk c+1 is
interleaved with attention for chunk c at block granularity so the PE
stays continuously busy and reaches the full 2.4 GHz p-state):
  - x arrives bf16 (LN stats only) and host-pretransposed fp32 (xT, the
    QKV stationary operand); both DMA'd in 512-token chunks.
  - LN folded around QKV: one fused scalar_tensor_tensor per matmul
    region computes raw + mu*(-colsum(W)) (colsums negated on host);
    rstd folded into the rotary cos/sin tiles and the V copy.
  - rotary on DVE (pair-interleaved features, adjacent-pair swap view).
  - attention: S^T = K_j Q_c^T blocks, trimmed to the causal range
    [q0:512]; exp on ACT; causal mask via one gpsimd affine_select on
    just the diagonal 128x128 square; PV in bf16 with a ones column on
    V so the softmax denominator comes out of the same matmul;
    normalization via DVE reciprocal + gpsimd broadcast.
  - PSUM packed into exactly 8 banks: qk[1], {v,transpose,y} shared
    ring[1], S double-buffered[4], PV accumulator[2].
"""
import sys
import os
import numpy as np
from contextlib import ExitStack

sys.path.insert(0, '/opt/trn_rl_repo')
if '/root/.axon_site' not in sys.path:
    sys.path.insert(0, '/root/.axon_site')

import concourse.bass as bass
import concourse.tile as tile
from concourse import mybir, bacc
from concourse.bass_utils import run_bass_kernel_spmd
from concourse.masks import make_identity

F32 = mybir.dt.float32
F32R = mybir.dt.float32r
BF16 = mybir.dt.bfloat16
EXPF = mybir.ActivationFunctionType.Exp
LNF = mybir.ActivationFunctionType.Ln
MUL = mybir.AluOpType.mult
ADD = mybir.AluOpType.add

N = 2048          # sequence length
D = 1024          # model dim
DH = 64           # head dim
NT = N // 128     # 16 token tiles
NCH = N // 512    # 4 q-chunks
LN_EPS = 1e-5

_cache = {}


def _patch_act_tables():
    """Keep Exp and Ln only in natural_log_exp_and_others so the table-load
    inserter can't ping-pong between exp_and_others and natural_log."""
    if _cache.get('act_patched'):
        return
    import concourse.bacc as bacc_mod
    orig = bacc_mod.get_activation_tables

    def patched(arch):
        t = dict(orig(arch))
        out = {}
        for name, fns in t.items():
            fns = set(fns)
            if name != 'natural_log_exp_and_others':
                fns.discard(mybir.ActivationFunctionType.Exp)
                fns.discard(mybir.ActivationFunctionType.Ln)
            out[name] = fns
        return out

    bacc_mod.get_activation_tables = patched
    _cache['act_patched'] = True


def _ap(t, off, dims):
    """Free-dim view of tile t at free-offset off with custom free dims."""
    return bass.AP(tensor=t.tensor, offset=t.offset + off, ap=[t.ap[0]] + dims)


def build():
    _patch_act_tables()
    nc = bacc.Bacc()
    x_d = nc.declare_dram_parameter("x", [N, D], BF16, isOutput=False)
    xT_d = nc.declare_dram_parameter("xT", [D, N], F32R, isOutput=False)
    wqk_d = nc.declare_dram_parameter("wqk", [D, 512], F32R, isOutput=False)
    wv_d = nc.declare_dram_parameter("wv", [D, 256], F32R, isOutput=False)
    wo_d = nc.declare_dram_parameter("wo", [256, D], BF16, isOutput=False)
    trig_d = nc.declare_dram_parameter("trig", [N, 2 * DH], F32, isOutput=False)
    cqkv_d = nc.declare_dram_parameter("cqkv", [1, 768], F32, isOutput=False)
    y_d = nc.declare_dram_parameter("y", [N, D], F32, isOutput=True)

    x_r = x_d.rearrange("(j p) d -> p j d", p=128)
    xT_r = xT_d.rearrange("(k p) t -> p k t", p=128)
    y_r = y_d.rearrange("(j p) d -> p j d", p=128)

    with tile.TileContext(nc) as tc:
        with ExitStack() as cx:
            const = cx.enter_context(tc.tile_pool(name="const", bufs=1))
            xp = cx.enter_context(tc.tile_pool(name="xp", bufs=2))
            pa = cx.enter_context(tc.tile_pool(name="pa", bufs=2))
            st = cx.enter_context(tc.tile_pool(name="st", bufs=4))
            pb = cx.enter_context(tc.tile_pool(name="pb", bufs=6))
            nrm = cx.enter_context(tc.tile_pool(name="nrm", bufs=2))
            yout = cx.enter_context(tc.tile_pool(name="yout", bufs=1))
            # PSUM: exactly 8 banks
            pq = cx.enter_context(tc.tile_pool(name="pq", bufs=1, space="PSUM"))
            pm = cx.enter_context(tc.tile_pool(name="pm", bufs=1, space="PSUM"))
            ps = cx.enter_context(tc.tile_pool(name="ps", bufs=2, space="PSUM"))
            pot = cx.enter_context(tc.tile_pool(name="pot", bufs=1, space="PSUM"))

            # --- chunked input DMAs (issued up front, ring-buffered,
            # spread across engine DGE queues so transfers parallelize) ----
            x_ch, xT_ch = [], []
            wqk = const.tile([128, 8, 512], F32R)
            wv = const.tile([128, 8, 256], F32R)
            wo = const.tile([128, 2, 1024], BF16)
            trig = const.tile([128, NT, 2 * DH], F32)
            cqkv_row = const.tile([1, 768], F32)

            def dma_x(c, eng):
                xt = xp.tile([128, 4, D], BF16, tag="x", name="x_t")
                nc.scalar.dma_start(out=xt[:], in_=x_r[:, 4 * c:4 * c + 4, :])
                x_ch.append(xt)
                xtt = xp.tile([128, 8, 512], F32R, tag="xT", name="xT_t")
                eng.dma_start(out=xtt[:], in_=xT_r[:, :, 512 * c:512 * (c + 1)])
                xT_ch.append(xtt)

            nc.gpsimd.dma_start(out=wqk[:], in_=wqk_d.rearrange("(k p) f -> p k f", p=128))
            nc.gpsimd.dma_start(out=cqkv_row[:], in_=cqkv_d[:])
            dma_x(0, nc.sync)
            nc.gpsimd.dma_start(out=trig[:],
                                in_=trig_d.rearrange("(j p) d -> p j d", p=128))
            nc.gpsimd.dma_start(out=wv[:], in_=wv_d.rearrange("(k p) f -> p k f", p=128))
            dma_x(1, nc.sync)
            nc.gpsimd.dma_start(out=wo[:], in_=wo_d.rearrange("(g p) f -> p g f", p=128))
            dma_x(2, nc.sync)
            dma_x(3, nc.sync)

            ident = const.tile([128, 128], F32)
            make_identity(nc, ident[:])
            eps_t = const.tile([128, 1], F32)
            nc.vector.memset(eps_t[:], LN_EPS)
            cqkv_b = const.tile([128, 768], F32)
            nc.gpsimd.partition_broadcast(cqkv_b[:], cqkv_row[:])

            # persistent activations.  qkT packs qT and kT in one tile
            # ([2 heads x 64 d, {q pair0, q pair1, k pair0, k pair1}, tok])
            # so one DVE copy per tile moves all four transposed strips.
            qkT = const.tile([128, 4, N], BF16)
            vA = const.tile([128, NT, 4, DH + 1], BF16)   # V_ext, ones col 64
            oT = const.tile([128, 2, N], BF16)     # attention out^T per pair
            nc.gpsimd.memset(vA[:, :, :, DH:DH + 1], 1.0)

            rot = {}     # tile j -> qk_rot handle (consumed by transposes)

            # ---------------- tile production -----------------------------
            def emit_tp(jp):
                """Transpose tile jp's rotated q/k into qkT."""
                sl = slice(128 * jp, 128 * (jp + 1))
                tp = pm.tile([128, 4, 128], F32, tag="m", name="tp")
                for f in range(4):
                    nc.tensor.transpose(tp[:, f, :],
                                        rot[jp][:, 128 * f:128 * (f + 1)], ident[:])
                nc.vector.tensor_copy(out=qkT[:, :, sl], in_=tp[:])
                del rot[jp]

            def produce(j):
                c, jl = j // 4, j % 4
                if j >= 1:
                    emit_tp(j - 1)
                x_t = x_ch[c]
                xT_t = xT_ch[c]
                qk_ps = pq.tile([128, 512], F32, tag="qk", name="qkp")
                for k in range(8):
                    nc.tensor.matmul(qk_ps[:], xT_t[:, k, 128 * jl:128 * (jl + 1)],
                                     wqk[:, k, :], start=(k == 0), stop=(k == 7))
                v_ps = pm.tile([128, 256], F32, tag="m", name="vp")
                for k in range(8):
                    nc.tensor.matmul(v_ps[:], xT_t[:, k, 128 * jl:128 * (jl + 1)],
                                     wv[:, k, :], start=(k == 0), stop=(k == 7))
                stats = st.tile([128, 2, 6], F32, tag="stats", name="stats")
                nc.vector.bn_stats(out=stats[:, 0, :], in_=x_t[:, jl, 0:512])
                nc.vector.bn_stats(out=stats[:, 1, :], in_=x_t[:, jl, 512:1024])
                mv = st.tile([128, 2], F32, tag="mv", name="mv")
                nc.vector.bn_aggr(out=mv[:], in_=stats[:])
                # rstd = exp(-0.5*ln(var+eps))
                lnv = st.tile([128, 1], F32, tag="lnv", name="lnv")
                nc.scalar.activation(out=lnv[:], in_=mv[:, 1:2], func=LNF, bias=eps_t[:])
                rstd = st.tile([128, 1], F32, tag="rstd", name="rstd")
                nc.scalar.activation(out=rstd[:], in_=lnv[:], func=EXPF, scale=-0.5)
                # LN correction fused: qkv_c = raw + mu*(-colsum) (cqkv negated)
                qkc = pa.tile([128, 512], F32, tag="qkc", name="qkc")
                nc.vector.scalar_tensor_tensor(
                    out=qkc[:], in0=cqkv_b[:, 0:512], scalar=mv[:, 0:1],
                    in1=qk_ps[:], op0=MUL, op1=ADD)
                vc = pa.tile([128, 256], F32, tag="vc", name="vc")
                nc.vector.scalar_tensor_tensor(
                    out=vc[:], in0=cqkv_b[:, 512:768], scalar=mv[:, 0:1],
                    in1=v_ps[:], op0=MUL, op1=ADD)
                nc.vector.tensor_scalar(
                    out=vA[:, j, :, 0:DH],
                    in0=vc[:].rearrange("p (h d) -> p h d", d=DH),
                    scalar1=rstd[:], scalar2=None, op0=MUL)
                # rstd-scaled rotary coefficient tiles (cos|sin packed)
                css = st.tile([128, 2 * DH], F32, tag="css", name="css")
                nc.vector.tensor_scalar(out=css[:], in0=trig[:, j, :],
                                        scalar1=rstd[:], scalar2=None, op0=MUL)
                # rotary: qk_rot = qkc*cos + swap_adj(qkc)*sin
                cos_b = _ap(css, 0, [[0, 8], [1, DH]])
                sin_b = _ap(css, DH, [[0, 8], [2, 32], [1, 2]])
                t_cos = pa.tile([128, 512], F32, tag="tcos", name="tcos")
                nc.vector.tensor_tensor(
                    out=t_cos[:].rearrange("p (g d) -> p g d", d=DH),
                    in0=qkc[:].rearrange("p (g d) -> p g d", d=DH),
                    in1=cos_b, op=MUL)
                t_sin = pa.tile([128, 512], F32, tag="tsin", name="tsin")
                qk_swap = _ap(qkc, 1, [[DH, 8], [2, 32], [-1, 2]])
                nc.vector.tensor_tensor(
                    out=t_sin[:].rearrange("p (g i t) -> p g i t", g=8, t=2),
                    in0=qk_swap, in1=sin_b, op=MUL)
                qk_rot = pa.tile([128, 512], F32, tag="qkr", name="qkr")
                nc.vector.tensor_tensor(out=qk_rot[:], in0=t_cos[:], in1=t_sin[:],
                                        op=ADD)
                rot[j] = qk_rot

            # ---------------- attention closures --------------------------
            def attention_closures(c):
                """Returns (items, fins): per-block S/exp/mask/PV closures
                with a fast PSUM->SBUF drain at each head-pair end, plus
                deferred normalize-finalize closures (emitted later so the
                ACT queue stays pure-exp while attention is running)."""
                njb = 4 * c + 4
                items, fins = [], []
                for hp in range(2):
                    hst = {}

                    def emit_pv(pj, pt, hp=hp, hst=hst, njb=njb, c=c):
                        pq0 = max(0, 128 * (pj - 4 * c))
                        otp = hst['ot']
                        for hh in range(2):
                            nc.tensor.matmul(
                                otp[:, 512 * hh + pq0:512 * (hh + 1)],
                                vA[:, pj, 2 * hp + hh, :],
                                pt[:, hh, pq0:512],
                                start=(pj == 0), stop=(pj == njb - 1),
                                skip_group_check=True)

                    def blk(jj, hp=hp, hst=hst, c=c, emit_pv=emit_pv):
                        if jj == 0:
                            hst['ot'] = pot.tile([DH + 1, 1024], F32, tag="ot",
                                                 name="ot")
                            hst['pend'] = []
                        dj = jj - 4 * c
                        q0 = max(0, 128 * dj)
                        s_ps = ps.tile([128, 2, 512], F32, tag="s", name="s")
                        for hh in range(2):
                            bp = 64 * hh
                            nc.tensor.matmul(
                                s_ps[:, hh, q0:512],
                                qkT[bp:bp + 64, 2 + hp, 128 * jj:128 * (jj + 1)],
                                qkT[bp:bp + 64, hp, 512 * c + q0:512 * (c + 1)],
                                start=True, stop=True, skip_group_check=True)
                        p_t = pb.tile([128, 2, 512], BF16, tag="p", name="p")
                        if dj < 0:
                            nc.scalar.activation(out=p_t[:], in_=s_ps[:], func=EXPF)
                        else:
                            nc.scalar.activation(out=p_t[:, :, q0:512],
                                                 in_=s_ps[:, :, q0:512], func=EXPF)
                            nc.gpsimd.affine_select(
                                out=p_t[:, :, q0:q0 + 128],
                                in_=p_t[:, :, q0:q0 + 128],
                                compare_op=mybir.AluOpType.is_ge,
                                fill=0.0, base=0,
                                pattern=[[0, 2], [1, 128]], channel_multiplier=-1)
                        hst['pend'].append((jj, p_t))
                        if len(hst['pend']) > 2:
                            emit_pv(*hst['pend'].pop(0))

                    def norm_copy(hp=hp, hst=hst, c=c, emit_pv=emit_pv):
                        while hst['pend']:
                            emit_pv(*hst['pend'].pop(0))
                        otp = hst['ot']
                        # drain PSUM->SBUF on DVE only, so the ot slot frees
                        # fast without touching the exp-busy ACT queue
                        oraw = nrm.tile([DH + 1, 1024], F32, tag="oraw",
                                        name="oraw")
                        nc.vector.tensor_copy(out=oraw[:], in_=otp[:])
                        hst['oraw'] = oraw

                    def norm_fin(hp=hp, hst=hst, c=c):
                        oraw = hst['oraw']
                        lnl = nrm.tile([1, 1024], F32, tag="lnl", name="lnl", bufs=1)
                        nc.scalar.activation(out=lnl[:], in_=oraw[DH:DH + 1, :],
                                             func=LNF)
                        rec = nrm.tile([1, 1024], F32, tag="rec", name="rec", bufs=1)
                        nc.scalar.activation(out=rec[:], in_=lnl[:], func=EXPF,
                                             scale=-1.0)
                        rec_b = nrm.tile([64, 1024], F32, tag="recb", name="recb", bufs=1)
                        nc.gpsimd.partition_broadcast(rec_b[:], rec[:])
                        for hh in range(2):
                            nc.gpsimd.tensor_tensor(
                                out=oT[64 * hh:64 * (hh + 1), hp,
                                       512 * c:512 * (c + 1)],
                                in0=oraw[0:DH, 512 * hh:512 * (hh + 1)],
                                in1=rec_b[:, 512 * hh:512 * (hh + 1)],
                                op=MUL)

                    for jj in range(njb):
                        items.append(lambda jj=jj, blk=blk: blk(jj))
                    items.append(norm_copy)
                    fins.append(norm_fin)
                return items, fins

            # ---------------- out-projection ------------------------------
            def outproj(c):
                # the last chunk's out-projection runs after all attention:
                # draw its psum from the then-idle double-buffered s ring so
                # matmul/copy overlap instead of serializing on the misc slot
                ypool, ytag = (ps, "s") if c == 3 else (pm, "m")
                ysb = yout.tile([128, 4, 1024], F32, tag="ysb", name="ysb")
                for jl in range(4):
                    j = 4 * c + jl
                    for m in range(2):
                        y_ps = ypool.tile([128, 512], F32, tag=ytag, name="yp")
                        for hp2 in range(2):
                            nc.tensor.matmul(y_ps[:],
                                             oT[:, hp2, 128 * j:128 * (j + 1)],
                                             wo[:, hp2, 512 * m:512 * (m + 1)],
                                             start=(hp2 == 0), stop=(hp2 == 1))
                        dst = ysb[:, jl, 512 * m:512 * (m + 1)]
                        if (jl + m) % 2 == 0:
                            nc.vector.tensor_copy(out=dst, in_=y_ps[:])
                        else:
                            nc.scalar.copy(out=dst, in_=y_ps[:])
                nc.sync.dma_start(out=y_r[:, 4 * c:4 * c + 4, :], in_=ysb[:])

            # ---------------- pipelined emission --------------------------
            fins_carry = []
            for step in range(5):
                cprev = step - 1
                blocks, fins = attention_closures(cprev) if cprev >= 0 else ([], [])
                bidx = 0

                def drain(n):
                    nonlocal bidx
                    for _ in range(n):
                        if bidx < len(blocks):
                            blocks[bidx]()
                            bidx += 1

                if step < 4:
                    quarter = (len(blocks) + 3) // 4 if blocks else 0
                    for i, j in enumerate(range(4 * step, 4 * step + 4)):
                        produce(j)
                        if i == 0 and fins_carry:
                            for f in fins_carry:
                                f()
                        if i == 1 and step >= 2:
                            outproj(step - 2)
                        drain(quarter)
                    drain(len(blocks))
                else:
                    emit_tp(15)
                    for f in fins_carry:
                        f()
                    outproj(2)
                    drain(len(blocks))
                fins_carry = fins
            for f in fins_carry:
                f()
            outproj(3)

    nc.finalize()
    return nc


def _host_shards(x, rotary_pos_emb, ln_w, ln_b, w_qkv, w_out):
    """Build the 8 per-core input maps."""
    import ml_dtypes
    BF = ml_dtypes.bfloat16
    SCALE = DH ** -0.5
    # pair-interleaved feature order within each head: (i, i+32) adjacent
    perm = np.empty(DH, dtype=np.int64)
    perm[0::2] = np.arange(32)
    perm[1::2] = np.arange(32) + 32
    cos = np.cos(rotary_pos_emb).astype(np.float32)     # [N, DH]
    sin = np.sin(rotary_pos_emb).astype(np.float32)
    cosn = cos[:, perm]
    sinn = sin[:, perm].copy()
    sinn[:, 0::2] *= -1.0                               # -sin on even slots
    trig = np.ascontiguousarray(np.concatenate([cosn, sinn], axis=1))

    lw = np.asarray(ln_w, dtype=np.float32)[:, None]
    w_q = (np.asarray(w_qkv[:, 0:1024]) * SCALE * lw).astype(np.float32)
    w_k = (np.asarray(w_qkv[:, 1024:2048]) * lw).astype(np.float32)
    w_v = (np.asarray(w_qkv[:, 2048:3072]) * lw).astype(np.float32)
    if np.abs(np.asarray(ln_b)).max() != 0:
        raise NotImplementedError("nonzero ln_b not supported by this kernel")

    in_maps = []
    for core in range(8):
        bi = core // 4
        h0 = 4 * (core % 4)
        qcols = [w_q[:, DH * (h0 + h):DH * (h0 + h + 1)][:, perm] for h in range(4)]
        kcols = [w_k[:, DH * (h0 + h):DH * (h0 + h + 1)][:, perm] for h in range(4)]
        wqk = np.ascontiguousarray(np.concatenate(qcols + kcols, axis=1))
        wv = np.ascontiguousarray(w_v[:, DH * h0:DH * (h0 + 4)])
        wo = np.ascontiguousarray(
            np.asarray(w_out)[DH * h0:DH * (h0 + 4), :]).astype(BF)
        xb = np.ascontiguousarray(np.asarray(x[bi])).astype(np.float32)
        # correction colsums NEGATED so the fused stt is (c*mu + raw)
        cq = -np.concatenate([wqk.sum(axis=0), wv.sum(axis=0)]).astype(np.float32)
        in_maps.append({
            "x": np.ascontiguousarray(xb).astype(BF),
            "xT": np.ascontiguousarray(xb.T),
            "wqk": wqk, "wv": wv, "wo": wo,
            "trig": trig,
            "cqkv": np.ascontiguousarray(cq[None, :]),
        })
    return in_maps


def run(inputs, trace=False):
    if 'nc' not in _cache:
        _cache['nc'] = build()
    nc = _cache['nc']
    in_maps = _host_shards(**inputs)
    res = run_bass_kernel_spmd(nc, in_maps, core_ids=list(range(8)), trace=trace)
    parts = [res.results[i]["y"] for i in range(8)]
    y = np.stack([
        parts[0] + parts[1] + parts[2] + parts[3],
        parts[4] + parts[5] + parts[6] + parts[7],
    ]).astype(np.float32)
    return y, res


def kernel(**inputs):
    y, _ = run(inputs, trace=False)
    return y


# revision 25
# speedup vs baseline: 1.2797x; 1.1952x over previous
"""Causal multi-head attention block (LN + rotary QKV + causal attention +
out-projection) on 8 Trainium2 NeuronCores.

Sharding: data-parallel over batch (b=2), tensor-parallel over heads
(16 heads -> 4 per core). Core c handles batch c//4, heads 4*(c%4)..+4.
Each core computes a partial out-projection (row-parallel w_out); the host
sums the 4 partials per batch.

Per-core pipeline (software-pipelined: tile production for chunk c+1 is
interleaved with attention for chunk c at block granularity so the PE
stays continuously busy and reaches the full 2.4 GHz p-state):
  - x arrives bf16 (LN stats only) and host-pretransposed fp32 (xT, the
    QKV stationary operand); both DMA'd in 512-token chunks.
  - LN folded around QKV: one fused scalar_tensor_tensor per matmul
    region computes raw + mu*(-colsum(W)) (colsums negated on host);
    rstd folded into the rotary cos/sin tiles and the V copy.
  - rotary on DVE (pair-interleaved features, adjacent-pair swap view).
  - attention: S^T = K_j Q_c^T blocks, trimmed to the causal range
    [q0:512]; exp on ACT; causal mask via one gpsimd affine_select on
    just the diagonal 128x128 square; PV in bf16 with a ones column on
    V so the softmax denominator comes out of the same matmul;
    normalization via DVE reciprocal + gpsimd broadcast.
  - PSUM packed into exactly 8 banks: qk[1], {v,transpose,y} shared
    ring[1], S double-buffered[4], PV accumulator[2].
"""
import sys
import os
import numpy as np
from contextlib import ExitStack

sys.path.insert(0, '/opt/trn_rl_repo')
if '/root/.axon_site' not in sys.path:
    sys.path.insert(0, '/root/.axon_site')

import concourse.bass as bass
import concourse.tile as tile
from concourse import mybir, bacc
from concourse.bass_utils import run_bass_kernel_spmd
from concourse.masks import make_identity

F32 = mybir.dt.float32
F32R = mybir.dt.float32r
BF16 = mybir.dt.bfloat16
EXPF = mybir.ActivationFunctionType.Exp
LNF = mybir.ActivationFunctionType.Ln
MUL = mybir.AluOpType.mult
ADD = mybir.AluOpType.add

N = 2048          # sequence length
D = 1024          # model dim
DH = 64           # head dim
NT = N // 128     # 16 token tiles
NCH = N // 512    # 4 q-chunks
LN_EPS = 1e-5

_cache = {}


def _patch_act_tables():
    """Keep Exp and Ln only in natural_log_exp_and_others so the table-load
    inserter can't ping-pong between exp_and_others and natural_log."""
    if _cache.get('act_patched'):
        return
    import concourse.bacc as bacc_mod
    orig = bacc_mod.get_activation_tables

    def patched(arch):
        t = dict(orig(arch))
        out = {}
        for name, fns in t.items():
            fns = set(fns)
            if name != 'natural_log_exp_and_others':
                fns.discard(mybir.ActivationFunctionType.Exp)
                fns.discard(mybir.ActivationFunctionType.Ln)
            out[name] = fns
        return out

    bacc_mod.get_activation_tables = patched
    _cache['act_patched'] = True


def _ap(t, off, dims):
    """Free-dim view of tile t at free-offset off with custom free dims."""
    return bass.AP(tensor=t.tensor, offset=t.offset + off, ap=[t.ap[0]] + dims)


def build():
    _patch_act_tables()
    nc = bacc.Bacc()
    x_d = nc.declare_dram_parameter("x", [N, D], BF16, isOutput=False)
    xT_d = nc.declare_dram_parameter("xT", [D, N], BF16, isOutput=False)
    wqk_d = nc.declare_dram_parameter("wqk", [D, 512], BF16, isOutput=False)
    wv_d = nc.declare_dram_parameter("wv", [D, 256], BF16, isOutput=False)
    wo_d = nc.declare_dram_parameter("wo", [256, D], BF16, isOutput=False)
    trig_d = nc.declare_dram_parameter("trig", [N, 2 * DH], F32, isOutput=False)
    cqkv_d = nc.declare_dram_parameter("cqkv", [1, 768], F32, isOutput=False)
    y_d = nc.declare_dram_parameter("y", [N, D], F32, isOutput=True)

    x_r = x_d.rearrange("(j p) d -> p j d", p=128)
    xT_r = xT_d.rearrange("(k p) t -> p k t", p=128)
    y_r = y_d.rearrange("(j p) d -> p j d", p=128)

    with tile.TileContext(nc) as tc:
        with ExitStack() as cx:
            const = cx.enter_context(tc.tile_pool(name="const", bufs=1))
            xp = cx.enter_context(tc.tile_pool(name="xp", bufs=2))
            pa = cx.enter_context(tc.tile_pool(name="pa", bufs=3))
            st = cx.enter_context(tc.tile_pool(name="st", bufs=4))
            pb = cx.enter_context(tc.tile_pool(name="pb", bufs=6))
            nrm = cx.enter_context(tc.tile_pool(name="nrm", bufs=2))
            yout = cx.enter_context(tc.tile_pool(name="yout", bufs=1))
            # PSUM: exactly 8 banks
            pq = cx.enter_context(tc.tile_pool(name="pq", bufs=1, space="PSUM"))
            pm = cx.enter_context(tc.tile_pool(name="pm", bufs=1, space="PSUM"))
            ps = cx.enter_context(tc.tile_pool(name="ps", bufs=2, space="PSUM"))
            pot = cx.enter_context(tc.tile_pool(name="pot", bufs=1, space="PSUM"))

            # --- chunked input DMAs (issued up front, ring-buffered,
            # spread across engine DGE queues so transfers parallelize) ----
            x_ch, xT_ch = [], []
            wqk = const.tile([128, 8, 512], BF16)
            wv = const.tile([128, 8, 256], BF16)
            wo = const.tile([128, 2, 1024], BF16)
            trig = const.tile([128, NT, 2 * DH], F32)
            cqkv_row = const.tile([1, 768], F32)

            wqk_r = wqk_d.rearrange("(k p) f -> p k f", p=128)

            def dma_x(c, halves=False):
                xt = xp.tile([128, 4, D], BF16, tag="x", name="x_t")
                if halves:
                    nc.scalar.dma_start(out=xt[:, 0:2, :],
                                        in_=x_r[:, 4 * c:4 * c + 2, :])
                    nc.scalar.dma_start(out=xt[:, 2:4, :],
                                        in_=x_r[:, 4 * c + 2:4 * c + 4, :])
                else:
                    nc.scalar.dma_start(out=xt[:], in_=x_r[:, 4 * c:4 * c + 4, :])
                x_ch.append(xt)
                xtt = xp.tile([128, 8, 512], BF16, tag="xT", name="xT_t")
                base = 512 * c
                if halves:
                    nc.sync.dma_start(out=xtt[:, :, 0:256],
                                      in_=xT_r[:, :, base:base + 256])
                    nc.sync.dma_start(out=xtt[:, :, 256:512],
                                      in_=xT_r[:, :, base + 256:base + 512])
                else:
                    nc.sync.dma_start(out=xtt[:], in_=xT_r[:, :, base:base + 512])
                xT_ch.append(xtt)

            nc.gpsimd.dma_start(out=wqk[:, 0:4, :], in_=wqk_r[:, 0:4, :])
            nc.gpsimd.dma_start(out=cqkv_row[:], in_=cqkv_d[:])
            dma_x(0, halves=True)
            nc.gpsimd.dma_start(out=wqk[:, 4:8, :], in_=wqk_r[:, 4:8, :])
            nc.gpsimd.dma_start(out=trig[:],
                                in_=trig_d.rearrange("(j p) d -> p j d", p=128))
            nc.gpsimd.dma_start(out=wv[:], in_=wv_d.rearrange("(k p) f -> p k f", p=128))
            dma_x(1)
            nc.gpsimd.dma_start(out=wo[:], in_=wo_d.rearrange("(g p) f -> p g f", p=128))
            dma_x(2)
            dma_x(3)

            ident = const.tile([128, 128], F32)
            make_identity(nc, ident[:])
            eps_t = const.tile([128, 1], F32)
            nc.vector.memset(eps_t[:], LN_EPS)
            cqkv_b = const.tile([128, 768], F32)
            nc.gpsimd.partition_broadcast(cqkv_b[:], cqkv_row[:])

            # persistent activations.  qkT packs qT and kT in one tile
            # ([2 heads x 64 d, {q pair0, q pair1, k pair0, k pair1}, tok])
            # so one DVE copy per tile moves all four transposed strips.
            qkT = const.tile([128, 4, N], BF16)
            vA = const.tile([128, NT, 4, DH + 1], BF16)   # V_ext, ones col 64
            oT = const.tile([128, 2, N], BF16)     # attention out^T per pair
            nc.gpsimd.memset(vA[:, :, :, DH:DH + 1], 1.0)

            rot = {}     # tile j -> qk_rot handle (consumed by transposes)

            # ---------------- tile production -----------------------------
            def emit_tp(jp):
                """Transpose tile jp's rotated q/k into qkT."""
                sl = slice(128 * jp, 128 * (jp + 1))
                tp = pm.tile([128, 4, 128], F32, tag="m", name="tp")
                for f in range(4):
                    nc.tensor.transpose(tp[:, f, :],
                                        rot[jp][:, 128 * f:128 * (f + 1)], ident[:])
                if jp % 2 == 0:
                    nc.vector.tensor_copy(out=qkT[:, :, sl], in_=tp[:])
                else:
                    nc.scalar.copy(out=qkT[:, :, sl], in_=tp[:])
                del rot[jp]

            def produce(j):
                c, jl = j // 4, j % 4
                if j >= 1:
                    emit_tp(j - 1)
                x_t = x_ch[c]
                xT_t = xT_ch[c]
                qk_ps = pq.tile([128, 512], F32, tag="qk", name="qkp")
                for k in range(8):
                    nc.tensor.matmul(qk_ps[:], xT_t[:, k, 128 * jl:128 * (jl + 1)],
                                     wqk[:, k, :], start=(k == 0), stop=(k == 7))
                v_ps = pm.tile([128, 256], F32, tag="m", name="vp")
                for k in range(8):
                    nc.tensor.matmul(v_ps[:], xT_t[:, k, 128 * jl:128 * (jl + 1)],
                                     wv[:, k, :], start=(k == 0), stop=(k == 7))
                stats = st.tile([128, 2, 6], F32, tag="stats", name="stats")
                nc.vector.bn_stats(out=stats[:, 0, :], in_=x_t[:, jl, 0:512])
                nc.vector.bn_stats(out=stats[:, 1, :], in_=x_t[:, jl, 512:1024])
                mv = st.tile([128, 2], F32, tag="mv", name="mv")
                nc.vector.bn_aggr(out=mv[:], in_=stats[:])
                # rstd = exp(-0.5*ln(var+eps))
                lnv = st.tile([128, 1], F32, tag="lnv", name="lnv")
                nc.scalar.activation(out=lnv[:], in_=mv[:, 1:2], func=LNF, bias=eps_t[:])
                rstd = st.tile([128, 1], F32, tag="rstd", name="rstd")
                nc.scalar.activation(out=rstd[:], in_=lnv[:], func=EXPF, scale=-0.5)
                # LN correction fused: qkv_c = raw + mu*(-colsum) (cqkv negated)
                qkc = pa.tile([128, 512], F32, tag="qkc", name="qkc")
                nc.vector.scalar_tensor_tensor(
                    out=qkc[:], in0=cqkv_b[:, 0:512], scalar=mv[:, 0:1],
                    in1=qk_ps[:], op0=MUL, op1=ADD)
                vc = pa.tile([128, 256], F32, tag="vc", name="vc")
                nc.vector.scalar_tensor_tensor(
                    out=vc[:], in0=cqkv_b[:, 512:768], scalar=mv[:, 0:1],
                    in1=v_ps[:], op0=MUL, op1=ADD)
                nc.vector.tensor_scalar(
                    out=vA[:, j, :, 0:DH],
                    in0=vc[:].rearrange("p (h d) -> p h d", d=DH),
                    scalar1=rstd[:], scalar2=None, op0=MUL)
                # rstd-scaled rotary coefficient tiles (cos|sin packed)
                css = st.tile([128, 2 * DH], F32, tag="css", name="css")
                nc.vector.tensor_scalar(out=css[:], in0=trig[:, j, :],
                                        scalar1=rstd[:], scalar2=None, op0=MUL)
                # rotary: qk_rot = qkc*cos + swap_adj(qkc)*sin
                cos_b = _ap(css, 0, [[0, 8], [1, DH]])
                sin_b = _ap(css, DH, [[0, 8], [2, 32], [1, 2]])
                t_cos = pa.tile([128, 512], F32, tag="tcos", name="tcos")
                nc.vector.tensor_tensor(
                    out=t_cos[:].rearrange("p (g d) -> p g d", d=DH),
                    in0=qkc[:].rearrange("p (g d) -> p g d", d=DH),
                    in1=cos_b, op=MUL)
                t_sin = pa.tile([128, 512], F32, tag="tsin", name="tsin")
                qk_swap = _ap(qkc, 1, [[DH, 8], [2, 32], [-1, 2]])
                nc.vector.tensor_tensor(
                    out=t_sin[:].rearrange("p (g i t) -> p g i t", g=8, t=2),
                    in0=qk_swap, in1=sin_b, op=MUL)
                qk_rot = pa.tile([128, 512], F32, tag="qkr", name="qkr")
                nc.gpsimd.tensor_tensor(out=qk_rot[:], in0=t_cos[:], in1=t_sin[:],
                                        op=ADD)
                rot[j] = qk_rot

            # ---------------- attention closures --------------------------
            def attention_closures(c):
                """Returns (items, fins): per-block S/exp/mask/PV closures
                with a fast PSUM->SBUF drain at each head-pair end, plus
                deferred normalize-finalize closures (emitted later so the
                ACT queue stays pure-exp while attention is running)."""
                njb = 4 * c + 4
                items, fins = [], []
                for hp in range(2):
                    hst = {}

                    def emit_pv(pj, pt, hp=hp, hst=hst, njb=njb, c=c):
                        pq0 = max(0, 128 * (pj - 4 * c))
                        otp = hst['ot']
                        for hh in range(2):
                            nc.tensor.matmul(
                                otp[:, 512 * hh + pq0:512 * (hh + 1)],
                                vA[:, pj, 2 * hp + hh, :],
                                pt[:, hh, pq0:512],
                                start=(pj == 0), stop=(pj == njb - 1),
                                skip_group_check=True)

                    def blk(jj, hp=hp, hst=hst, c=c, emit_pv=emit_pv):
                        if jj == 0:
                            hst['ot'] = pot.tile([DH + 1, 1024], F32, tag="ot",
                                                 name="ot")
                            hst['pend'] = []
                        dj = jj - 4 * c
                        q0 = max(0, 128 * dj)
                        s_ps = ps.tile([128, 2, 512], F32, tag="s", name="s")
                        for hh in range(2):
                            bp = 64 * hh
                            nc.tensor.matmul(
                                s_ps[:, hh, q0:512],
                                qkT[bp:bp + 64, 2 + hp, 128 * jj:128 * (jj + 1)],
                                qkT[bp:bp + 64, hp, 512 * c + q0:512 * (c + 1)],
                                start=True, stop=True, skip_group_check=True)
                        p_t = pb.tile([128, 2, 512], BF16, tag="p", name="p")
                        if dj < 0:
                            nc.scalar.activation(out=p_t[:], in_=s_ps[:], func=EXPF)
                        else:
                            nc.scalar.activation(out=p_t[:, :, q0:512],
                                                 in_=s_ps[:, :, q0:512], func=EXPF)
                            nc.gpsimd.affine_select(
                                out=p_t[:, :, q0:q0 + 128],
                                in_=p_t[:, :, q0:q0 + 128],
                                compare_op=mybir.AluOpType.is_ge,
                                fill=0.0, base=0,
                                pattern=[[0, 2], [1, 128]], channel_multiplier=-1)
                        hst['pend'].append((jj, p_t))
                        if len(hst['pend']) > 3:
                            emit_pv(*hst['pend'].pop(0))

                    def norm_copy(hp=hp, hst=hst, c=c, emit_pv=emit_pv):
                        while hst['pend']:
                            emit_pv(*hst['pend'].pop(0))
                        otp = hst['ot']
                        # drain PSUM->SBUF on DVE only, so the ot slot frees
                        # fast without touching the exp-busy ACT queue
                        oraw = nrm.tile([DH + 1, 1024], F32, tag="oraw",
                                        name="oraw")
                        nc.scalar.copy(out=oraw[:, 0:512], in_=otp[:, 0:512])
                        nc.vector.tensor_copy(out=oraw[:, 512:1024],
                                              in_=otp[:, 512:1024])
                        hst['oraw'] = oraw

                    def norm_fin(hp=hp, hst=hst, c=c):
                        oraw = hst['oraw']
                        lnl = nrm.tile([1, 1024], F32, tag="lnl", name="lnl", bufs=1)
                        nc.scalar.activation(out=lnl[:], in_=oraw[DH:DH + 1, :],
                                             func=LNF)
                        rec = nrm.tile([1, 1024], F32, tag="rec", name="rec", bufs=1)
                        nc.scalar.activation(out=rec[:], in_=lnl[:], func=EXPF,
                                             scale=-1.0)
                        rec_b = nrm.tile([64, 1024], F32, tag="recb", name="recb", bufs=1)
                        nc.gpsimd.partition_broadcast(rec_b[:], rec[:])
                        for hh in range(2):
                            nc.vector.tensor_tensor(
                                out=oT[64 * hh:64 * (hh + 1), hp,
                                       512 * c:512 * (c + 1)],
                                in0=oraw[0:DH, 512 * hh:512 * (hh + 1)],
                                in1=rec_b[:, 512 * hh:512 * (hh + 1)],
                                op=MUL)

                    for jj in range(njb):
                        items.append(lambda jj=jj, blk=blk: blk(jj))
                    items.append(norm_copy)
                    fins.append(norm_fin)
                return items, fins

            # ---------------- out-projection ------------------------------
            def outproj(c):
                # the last chunk's out-projection runs after all attention:
                # draw its psum from the then-idle double-buffered s ring so
                # matmul/copy overlap instead of serializing on the misc slot
                ypool, ytag = (ps, "s") if c == 3 else (pm, "m")
                ysb = yout.tile([128, 4, 1024], F32, tag="ysb", name="ysb")
                for jl in range(4):
                    j = 4 * c + jl
                    for m in range(2):
                        y_ps = ypool.tile([128, 512], F32, tag=ytag, name="yp")
                        for hp2 in range(2):
                            nc.tensor.matmul(y_ps[:],
                                             oT[:, hp2, 128 * j:128 * (j + 1)],
                                             wo[:, hp2, 512 * m:512 * (m + 1)],
                                             start=(hp2 == 0), stop=(hp2 == 1))
                        dst = ysb[:, jl, 512 * m:512 * (m + 1)]
                        if (jl + m) % 2 == 0:
                            nc.vector.tensor_copy(out=dst, in_=y_ps[:])
                        else:
                            nc.scalar.copy(out=dst, in_=y_ps[:])
                nc.sync.dma_start(out=y_r[:, 4 * c:4 * c + 4, :], in_=ysb[:])

            # ---------------- pipelined emission --------------------------
            fins_carry = []
            for step in range(5):
                cprev = step - 1
                blocks, fins = attention_closures(cprev) if cprev >= 0 else ([], [])
                bidx = 0

                def drain(n):
                    nonlocal bidx
                    for _ in range(n):
                        if bidx < len(blocks):
                            blocks[bidx]()
                            bidx += 1

                if step < 4:
                    quarter = (len(blocks) + 3) // 4 if blocks else 0
                    for i, j in enumerate(range(4 * step, 4 * step + 4)):
                        produce(j)
                        if i == 0 and fins_carry:
                            for f in fins_carry:
                                f()
                        if i == 1 and step >= 2:
                            outproj(step - 2)
                        drain(quarter)
                    drain(len(blocks))
                else:
                    emit_tp(15)
                    for f in fins_carry:
                        f()
                    outproj(2)
                    drain(len(blocks))
                fins_carry = fins
            for f in fins_carry:
                f()
            outproj(3)

    nc.finalize()
    return nc


def _host_shards(x, rotary_pos_emb, ln_w, ln_b, w_qkv, w_out):
    """Build the 8 per-core input maps."""
    import ml_dtypes
    BF = ml_dtypes.bfloat16
    SCALE = DH ** -0.5
    # pair-interleaved feature order within each head: (i, i+32) adjacent
    perm = np.empty(DH, dtype=np.int64)
    perm[0::2] = np.arange(32)
    perm[1::2] = np.arange(32) + 32
    cos = np.cos(rotary_pos_emb).astype(np.float32)     # [N, DH]
    sin = np.sin(rotary_pos_emb).astype(np.float32)
    cosn = cos[:, perm]
    sinn = sin[:, perm].copy()
    sinn[:, 0::2] *= -1.0                               # -sin on even slots
    trig = np.ascontiguousarray(np.concatenate([cosn, sinn], axis=1))

    lw = np.asarray(ln_w, dtype=np.float32)[:, None]
    w_q = (np.asarray(w_qkv[:, 0:1024]) * SCALE * lw).astype(np.float32)
    w_k = (np.asarray(w_qkv[:, 1024:2048]) * lw).astype(np.float32)
    w_v = (np.asarray(w_qkv[:, 2048:3072]) * lw).astype(np.float32)
    if np.abs(np.asarray(ln_b)).max() != 0:
        raise NotImplementedError("nonzero ln_b not supported by this kernel")

    in_maps = []
    for core in range(8):
        bi = core // 4
        h0 = 4 * (core % 4)
        qcols = [w_q[:, DH * (h0 + h):DH * (h0 + h + 1)][:, perm] for h in range(4)]
        kcols = [w_k[:, DH * (h0 + h):DH * (h0 + h + 1)][:, perm] for h in range(4)]
        wqk = np.ascontiguousarray(np.concatenate(qcols + kcols, axis=1)).astype(BF)
        wv = np.ascontiguousarray(w_v[:, DH * h0:DH * (h0 + 4)]).astype(BF)
        wo = np.ascontiguousarray(
            np.asarray(w_out)[DH * h0:DH * (h0 + 4), :]).astype(BF)
        xb = np.ascontiguousarray(np.asarray(x[bi])).astype(np.float32)
        xbT_bf = np.ascontiguousarray(xb.T).astype(BF)
        # correction colsums NEGATED so the fused stt is (c*mu + raw);
        # computed from the bf16-rounded weights the matmul actually sees
        cq = -np.concatenate([wqk.astype(np.float32).sum(axis=0),
                              wv.astype(np.float32).sum(axis=0)])
        in_maps.append({
            "x": np.ascontiguousarray(xb).astype(BF),
            "xT": xbT_bf,
            "wqk": wqk, "wv": wv, "wo": wo,
            "trig": trig,
            "cqkv": np.ascontiguousarray(cq[None, :].astype(np.float32)),
        })
    return in_maps


def run(inputs, trace=False):
    if 'nc' not in _cache:
        _cache['nc'] = build()
    nc = _cache['nc']
    in_maps = _host_shards(**inputs)
    res = run_bass_kernel_spmd(nc, in_maps, core_ids=list(range(8)), trace=trace)
    parts = [res.results[i]["y"] for i in range(8)]
    y = np.stack([
        parts[0] + parts[1] + parts[2] + parts[3],
        parts[4] + parts[5] + parts[6] + parts[7],
    ]).astype(np.float32)
    return y, res


def kernel(**inputs):
    y, _ = run(inputs, trace=False)
    return y
